# revision 2
# baseline (speedup 1.0000x reference)
"""BaiChuan attention block on 8 Trainium2 NeuronCores.

Sharding: tensor-parallel over heads (4 heads/core) for QKV projection and
attention; AllGather of attention outputs (feature-major) per batch; o_proj
column-sharded (each core computes a 512-wide output-feature slice for all
tokens); host concatenates slices.

Precision: Q/K path in bf16 (softmax output is insensitive to Q/K rounding
since probabilities only depend on score *differences*, which are tiny here);
V path, attention values, and o_proj in float32r (full-rate fp32 matmul mode,
~2e-4 relative error).
"""
import numpy as np
import ml_dtypes

import concourse.bass as bass
import concourse.mybir as mybir
import concourse.tile as tile
from concourse import bacc, bass_utils

# Problem dims (hardcoded per contest contract)
B, S, H, NH = 2, 2048, 4096, 32
D = H // NH            # 128 head dim
CORES = 8
HPC = NH // CORES      # 4 heads per core
TOK = B * S            # 4096 tokens
FQ = HPC * D           # 512 per-core q/k/v feature width
TCW = 512              # token chunk width for QKV phase
NTC = TOK // TCW       # 8 chunks
HC = H // 128          # 32 contraction chunks
QB = 512               # attention q block
ROPE_THETA = 10000.0

F32 = mybir.dt.float32
F32R = mybir.dt.float32r
BF16 = mybir.dt.bfloat16

_CACHE = {}
LAST_RESULTS = None


def _build():
    nc = bacc.Bacc("TRN2", target_bir_lowering=False, debug=False, num_devices=CORES)

    x = nc.dram_tensor("x", [TOK, H], F32R, kind="ExternalInput").ap()
    wq = nc.dram_tensor("wq", [H, FQ], BF16, kind="ExternalInput").ap()
    wk = nc.dram_tensor("wk", [H, FQ], BF16, kind="ExternalInput").ap()
    wv = nc.dram_tensor("wv", [H, FQ], F32R, kind="ExternalInput").ap()
    wo = nc.dram_tensor("wo", [H, FQ], F32R, kind="ExternalInput").ap()
    cosq = nc.dram_tensor("cosq", [128, TOK], F32, kind="ExternalInput").ap()
    sinq = nc.dram_tensor("sinq", [128, TOK], F32, kind="ExternalInput").ap()
    cosk = nc.dram_tensor("cosk", [128, TOK], F32, kind="ExternalInput").ap()
    sink = nc.dram_tensor("sink", [128, TOK], F32, kind="ExternalInput").ap()
    masks = nc.dram_tensor("masks", [128, 4, QB], F32, kind="ExternalInput").ap()
    ones_col = nc.dram_tensor("ones_col", [128, 1], F32R, kind="ExternalInput").ap()
    ones_row = nc.dram_tensor("ones_row", [1, 128], F32R, kind="ExternalInput").ap()
    ident = nc.dram_tensor("ident", [128, 128], F32R, kind="ExternalInput").ap()
    out = nc.dram_tensor("out", [TOK, FQ], F32, kind="ExternalOutput").ap()

    with tile.TileContext(nc) as tc, nc.allow_low_precision(reason="f32r/bf16 kernel"):
        with tc.tile_pool(name="dram", bufs=1, space="DRAM") as dram, \
             tc.tile_pool(name="const", bufs=1) as constp:
            qt = dram.tile([FQ, TOK], BF16)      # Q^T (rope'd, pre-scaled)
            kt = dram.tile([FQ, TOK], BF16)      # K^T (rope'd)
            vv = dram.tile([TOK, FQ], F32R)      # V token-major
            aloc = [dram.tile([FQ, S], F32R, name=f"aloc{b_}") for b_ in range(B)]
            agth = [dram.tile([H, S], F32R, name=f"agth{b_}") for b_ in range(B)]

            ones_sb = constp.tile([128, 1], F32R)
            ones_row_sb = constp.tile([1, 128], F32R)
            ident_sb = constp.tile([128, 128], F32R)
            nc.sync.dma_start(ones_sb[:], ones_col)
            nc.sync.dma_start(ones_row_sb[:], ones_row)
            nc.sync.dma_start(ident_sb[:], ident)

            # ================= Phase 1: X^T + QKV projection =================
            with nc.named_scope("qkv"), \
                 tc.tile_pool(name="wqk", bufs=1) as wqkp, \
                 tc.tile_pool(name="wvs", bufs=3) as wvsp, \
                 tc.tile_pool(name="xtp", bufs=2) as xtp, \
                 tc.tile_pool(name="xin", bufs=3) as xinp, \
                 tc.tile_pool(name="xtr", bufs=3) as xtrp, \
                 tc.tile_pool(name="tab", bufs=2) as tabp, \
                 tc.tile_pool(name="qev", bufs=3) as evp, \
                 tc.tile_pool(name="ptr", bufs=2, space="PSUM") as ptrp, \
                 tc.tile_pool(name="pqk", bufs=2, space="PSUM") as pqkp, \
                 tc.tile_pool(name="pvp", bufs=1, space="PSUM") as pvp:

                wq_sb = wqkp.tile([128, HC, FQ], BF16)
                wk_sb = wqkp.tile([128, HC, FQ], BF16)
                nc.sync.dma_start(wq_sb[:], wq.rearrange("(k p) f -> p k f", p=128))
                nc.sync.dma_start(wk_sb[:], wk.rearrange("(k p) f -> p k f", p=128))

                for tci in range(NTC):
                    t0 = tci * TCW
                    cq = tabp.tile([128, TCW], F32, tag="cq")
                    sq_ = tabp.tile([128, TCW], F32, tag="sq")
                    ck = tabp.tile([128, TCW], F32, tag="ck")
                    sk_ = tabp.tile([128, TCW], F32, tag="sk")
                    nc.sync.dma_start(cq[:], cosq[:, t0:t0 + TCW])
                    nc.sync.dma_start(sq_[:], sinq[:, t0:t0 + TCW])
                    nc.sync.dma_start(ck[:], cosk[:, t0:t0 + TCW])
                    nc.sync.dma_start(sk_[:], sink[:, t0:t0 + TCW])

                    xt_bf = xtp.tile([128, HC, TCW], BF16, tag="xtbf")
                    pv_t = [pvp.tile([128, FQ], F32, tag=f"pv{ts}", name=f"pv{ts}")
                            for ts in range(4)]

                    for hc in range(HC):
                        xblk = xinp.tile([128, 4, 128], F32R, tag="xblk")
                        src = x[t0:t0 + TCW, hc * 128:(hc + 1) * 128]
                        nc.sync.dma_start(
                            xblk[:], src.rearrange("(a p) e -> p a e", p=128))
                        ptr_ = ptrp.tile([128, 4, 128], F32R, tag="ptr")
                        for ts in range(4):
                            nc.tensor.transpose(
                                ptr_[:, ts, :], xblk[:, ts, :], ident_sb[:])
                        nc.any.tensor_copy(
                            xt_bf[:, hc, :],
                            ptr_.rearrange("p a e -> p (a e)"))
                        xtr = xtrp.tile([128, 4, 128], F32R, tag="xtr")
                        nc.any.tensor_copy(xtr[:], ptr_[:])
                        wvs = wvsp.tile([128, FQ], F32R, tag="wvs")
                        nc.sync.dma_start(wvs[:], wv[hc * 128:(hc + 1) * 128, :])
                        for ts in range(4):
                            nc.tensor.matmul(
                                pv_t[ts][:], xtr[:, ts, :], wvs[:],
                                start=(hc == 0), stop=(hc == HC - 1))

                    # V eviction (token-major f32r)
                    for ts in range(4):
                        vout = evp.tile([128, FQ], F32R, tag="vout")
                        nc.any.tensor_copy(vout[:], pv_t[ts][:])
                        nc.sync.dma_start(
                            vv[t0 + ts * 128:t0 + (ts + 1) * 128, :], vout[:])

                    # Q^T / K^T with fused RoPE eviction
                    for f in range(8):
                        w_sb = wq_sb if f < 4 else wk_sb
                        fi = f % 4
                        pqk = pqkp.tile([128, TCW], F32, tag="pqk")
                        for hc in range(HC):
                            nc.tensor.matmul(
                                pqk[:], w_sb[:, hc, fi * 128:(fi + 1) * 128],
                                xt_bf[:, hc, :],
                                start=(hc == 0), stop=(hc == HC - 1))
                        cos_t = cq if f < 4 else ck
                        sin_t = sq_ if f < 4 else sk_
                        tmp = evp.tile([128, TCW], F32, tag="tmp")
                        sw = evp.tile([128, TCW], F32, tag="sw")
                        oev = evp.tile([128, TCW], BF16, tag="oev")
                        nc.vector.tensor_mul(tmp[:], pqk[:], cos_t[:])
                        nc.vector.tensor_mul(sw[0:64, :], pqk[64:128, :], sin_t[0:64, :])
                        nc.vector.tensor_mul(sw[64:128, :], pqk[0:64, :], sin_t[64:128, :])
                        nc.vector.tensor_add(oev[:], tmp[:], sw[:])
                        dst = qt if f < 4 else kt
                        nc.sync.dma_start(
                            dst[fi * 128:(fi + 1) * 128, t0:t0 + TCW], oev[:])

            # ============ Phase 2: attention + AllGather + o_proj ============
            with tc.tile_pool(name="wop", bufs=1) as wop, \
                 tc.tile_pool(name="mskp", bufs=1) as mskp, \
                 tc.tile_pool(name="akv", bufs=2) as akvp, \
                 tc.tile_pool(name="att", bufs=3) as attp, \
                 tc.tile_pool(name="oprj", bufs=2) as oprjp, \
                 tc.tile_pool(name="pat", bufs=1, space="PSUM") as patp, \
                 tc.tile_pool(name="pst2", bufs=2, space="PSUM") as pstp, \
                 tc.tile_pool(name="pfo", bufs=2, space="PSUM") as pfop:

                wo_sb = wop.tile([128, HC, FQ], F32R)
                nc.sync.dma_start(wo_sb[:], wo.rearrange("(k p) f -> p k f", p=128))
                mask_sb = mskp.tile([128, 4, QB], F32)
                nc.sync.dma_start(mask_sb[:], masks)

                with nc.named_scope("attn"):
                    for b_i in range(B):
                        for hl in range(HPC):
                            r0 = hl * 128
                            kt_sb = akvp.tile([128, S], BF16, tag="kt")
                            nc.sync.dma_start(
                                kt_sb[:], kt[r0:r0 + 128, b_i * S:(b_i + 1) * S])
                            v_sb = akvp.tile([128, S // 128, 128], F32R, tag="v")
                            nc.sync.dma_start(
                                v_sb[:],
                                vv[b_i * S:(b_i + 1) * S, r0:r0 + 128]
                                .rearrange("(c p) e -> p c e", p=128))
                            for j in range(S // QB):
                                q_sb = attp.tile([128, QB], BF16, tag="q")
                                nc.sync.dma_start(
                                    q_sb[:],
                                    qt[r0:r0 + 128,
                                       b_i * S + j * QB:b_i * S + (j + 1) * QB])
                                po = patp.tile([128, QB], F32, tag="po")
                                ps = patp.tile([1, QB], F32, tag="ps")
                                nkv = 4 * (j + 1)
                                for c in range(nkv):
                                    pst = pstp.tile([128, QB], F32, tag="pst")
                                    nc.tensor.matmul(
                                        pst[:], kt_sb[:, c * 128:(c + 1) * 128],
                                        q_sb[:], start=True, stop=True)
                                    dr = c - 4 * j
                                    pt = attp.tile([128, QB], F32R, tag="pt")
                                    if dr >= 0:
                                        et = attp.tile([128, QB], F32, tag="et")
                                        nc.scalar.activation(
                                            et[:], pst[:],
                                            mybir.ActivationFunctionType.Exp)
                                        nc.vector.tensor_mul(
                                            pt[:], et[:], mask_sb[:, dr, :])
                                    else:
                                        nc.scalar.activation(
                                            pt[:], pst[:],
                                            mybir.ActivationFunctionType.Exp)
                                    nc.tensor.matmul(
                                        po[:], v_sb[:, c, :], pt[:],
                                        start=(c == 0), stop=(c == nkv - 1))
                                    nc.tensor.matmul(
                                        ps[:], ones_sb[:], pt[:],
                                        start=(c == 0), stop=(c == nkv - 1))
                                r_sb = attp.tile([1, QB], F32R, tag="r")
                                nc.vector.reciprocal(r_sb[:], ps[:])
                                pb = pstp.tile([128, QB], F32, tag="pb")
                                nc.tensor.matmul(
                                    pb[:], ones_row_sb[:], r_sb[:],
                                    start=True, stop=True)
                                bsb = attp.tile([128, QB], F32, tag="bsb")
                                nc.vector.tensor_copy(bsb[:], pb[:])
                                o_sb = attp.tile([128, QB], F32R, tag="osb")
                                nc.vector.tensor_mul(o_sb[:], po[:], bsb[:])
                                nc.sync.dma_start(
                                    aloc[b_i][r0:r0 + 128, j * QB:(j + 1) * QB],
                                    o_sb[:])
                        nc.gpsimd.collective_compute(
                            "AllGather",
                            mybir.AluOpType.bypass,
                            ins=[aloc[b_i].opt()],
                            outs=[agth[b_i].opt()],
                            replica_groups=[list(range(CORES))],
                        )

                with nc.named_scope("oproj"):
                    for b_i in range(B):
                        for tt in range(S // 128):
                            lblk = oprjp.tile([128, HC, 128], F32R, tag="lblk")
                            nc.sync.dma_start(
                                lblk[:],
                                agth[b_i][:, tt * 128:(tt + 1) * 128]
                                .rearrange("(k p) e -> p k e", p=128))
                            pf = pfop.tile([128, FQ], F32, tag="pf")
                            for k in range(HC):
                                nc.tensor.matmul(
                                    pf[:], lblk[:, k, :], wo_sb[:, k, :],
                                    start=(k == 0), stop=(k == HC - 1))
                            fo = oprjp.tile([128, FQ], F32, tag="fo")
                            nc.any.tensor_copy(fo[:], pf[:])
                            nc.sync.dma_start(
                                out[b_i * S + tt * 128:b_i * S + (tt + 1) * 128, :],
                                fo[:])

    nc.compile()
    return nc


def _get_nc():
    if "nc" not in _CACHE:
        _CACHE["nc"] = _build()
    return _CACHE["nc"]


def kernel(positions, hidden_states, w_pack, w_o):
    global LAST_RESULTS
    nc = _get_nc()

    x = np.ascontiguousarray(
        np.asarray(hidden_states, dtype=np.float32).reshape(TOK, H))
    w_pack = np.asarray(w_pack, dtype=np.float32)
    w_o = np.asarray(w_o, dtype=np.float32)
    pos_flat = np.asarray(positions).reshape(-1).astype(np.float64)  # [TOK]

    half = D // 2
    inv = 1.0 / (ROPE_THETA ** (np.arange(half, dtype=np.float64) * 2.0 / D))
    f = np.outer(inv, pos_flat)                        # [64, TOK]
    cos = np.cos(f)
    sin = np.sin(f)
    cos_t = np.concatenate([cos, cos], axis=0)         # [128, TOK]
    sin_t = np.concatenate([-sin, sin], axis=0)
    scale = D ** -0.5
    cosq = (cos_t * scale).astype(np.float32)
    sinq = (sin_t * scale).astype(np.float32)
    cosk = cos_t.astype(np.float32)
    sink = sin_t.astype(np.float32)

    kvi = np.arange(128)[:, None, None]
    rr = np.arange(4)[None, :, None]
    qi = np.arange(QB)[None, None, :]
    masks = ((kvi + 128 * rr) <= qi).astype(np.float32)

    ones_col = np.ones((128, 1), np.float32)
    ones_row = np.ones((1, 128), np.float32)
    ident = np.eye(128, dtype=np.float32)

    in_maps = []
    for c in range(CORES):
        in_maps.append({
            "x": x,
            "wq": np.ascontiguousarray(
                w_pack[:, FQ * c:FQ * (c + 1)]).astype(ml_dtypes.bfloat16),
            "wk": np.ascontiguousarray(
                w_pack[:, H + FQ * c:H + FQ * (c + 1)]).astype(ml_dtypes.bfloat16),
            "wv": np.ascontiguousarray(w_pack[:, 2 * H + FQ * c:2 * H + FQ * (c + 1)]),
            "wo": np.ascontiguousarray(w_o[:, FQ * c:FQ * (c + 1)]),
            "cosq": cosq, "sinq": sinq, "cosk": cosk, "sink": sink,
            "masks": masks, "ones_col": ones_col, "ones_row": ones_row,
            "ident": ident,
        })

    res = bass_utils.run_bass_kernel_spmd(nc, in_maps, core_ids=list(range(CORES)))
    LAST_RESULTS = res
    outs = [res.results[c]["out"] for c in range(CORES)]
    return np.concatenate(outs, axis=1).reshape(B, S, H)
